# revision 2
# baseline (speedup 1.0000x reference)
"""Distributed Trainium2 Bass kernel for AltAttention, v2 (pipelined stream).

Sharding: batch x head-pair-group as v1. Core c handles batch c//2 and heads
6*(c%2) .. +6. Host sums the two partial outputs per batch (bproj/2 folded
into each partial).

v2 structural changes vs v1 (all bf16 math identical):
- q is processed in 512-wide blocks; per k-tile BOTH heads of a pair share one
  [128, 1024] PSUM scores tile (h0 cols 0:512, h1 cols 512:1024), so each kt
  needs ONE exp (ACT, FD=1024) and ONE ea-multiply (DVE, FD=1024).
- po accumulators are [65, 512] (1 PSUM bank each, tag bufs=3) so the next
  block's accumulation starts while the previous block normalizes.
- ea is stored host-side as contiguous [3, 4, 16, 128, 1024] tiles (256 KB per
  kt step), loaded alternately on the sync/gpsimd DMA queues.
- The normalize multiply runs on GPSIMD (SBUF-only operands), keeping DVE for
  the critical ea-multiply; the [65,512] PSUM->SBUF copy stays on ACT which
  frees the po slot quickly.
- qkv/v/proj matmuls are emitted interleaved with the attention stream as PE
  filler work; PSUM tags: sc 2x[128,1024] (4 banks) + po 3x[*,512] (3) +
  pq 1x[128,512] (1) = 8 banks.
"""
import sys

sys.path.insert(0, "/opt/trn_rl_repo")

import numpy as np
import ml_dtypes

import concourse.bass as bass
import concourse.mybir as mybir
import concourse.tile as tile
from concourse import bacc
from concourse.bass import ts
from concourse.bass_utils import run_bass_kernel_spmd

f32 = mybir.dt.float32
bf16 = mybir.dt.bfloat16
AF = mybir.ActivationFunctionType
OP = mybir.AluOpType

B, S, DIM, H = 4, 2048, 768, 12
HD = 64                 # head dim
HLOC = 6                # heads per core
SCALE = DIM ** (-0.5)   # note: module scales by full dim
P = 128
INF = DIM // P          # 6 input-feature chunks
NTT = S // P            # 16 token tiles
QB = 512                # query block
NQB = S // QB           # 4 query blocks
NKT = S // P            # 16 key tiles

_CACHED_NC = None


def _build():
    nc = bacc.Bacc("TRN2", target_bir_lowering=False, debug=False)

    xT_d = nc.declare_dram_parameter("xT", [P, INF, S], bf16, isOutput=False)
    wqk_d = nc.declare_dram_parameter("wqk", [P, INF, 6, P], bf16, isOutput=False)
    bqk_d = nc.declare_dram_parameter("bqk", [P, 6], f32, isOutput=False)
    wv_d = nc.declare_dram_parameter("wv", [P, INF, HLOC * HD], bf16, isOutput=False)
    bv_d = nc.declare_dram_parameter("bv", [1, HLOC * HD], f32, isOutput=False)
    wp_d = nc.declare_dram_parameter("wp", [P, 3, DIM], bf16, isOutput=False)
    bp2_d = nc.declare_dram_parameter("bp2", [1, DIM], f32, isOutput=False)
    ea_d = nc.declare_dram_parameter("ea", [3, NQB, NKT, P, 2 * QB], bf16,
                                     isOutput=False)
    out_d = nc.declare_dram_parameter("out", [S, DIM], bf16, isOutput=True)

    with tile.TileContext(nc) as tc:
        with (
            tc.tile_pool(name="persist", bufs=1) as persist,
            tc.tile_pool(name="stream", bufs=4) as stream,
            tc.tile_pool(name="norm", bufs=3) as norm,
            tc.tile_pool(name="dramp", bufs=3, space="DRAM") as dramp,
        ):
            # ---- persistent SBUF tensors ----
            xT = persist.tile([P, INF, S], bf16)
            wqk = persist.tile([P, INF, 6, P], bf16)
            bqk = persist.tile([P, 6], f32)
            wv = persist.tile([P, INF, HLOC * HD], bf16)
            bv_bc = persist.tile([P, HLOC * HD], f32)
            wp = persist.tile([P, 3, DIM], bf16)
            bp2_bc = persist.tile([P, DIM], f32)
            QK = persist.tile([P, 6, S], bf16)       # ch j: QT pair j; 3+j: KT
            VA = persist.tile([P, NTT, HLOC, HD + 1], bf16)
            xA = persist.tile([P, 3, S], bf16)       # attn out ^T, 2 heads/ch

            # initial loads in critical-path order: K-pair0 weights + first
            # token block first so the first qkv tile starts ~5us in
            nc.sync.dma_start(wqk[:, :, 3, :], wqk_d[:, :, 3, :])
            nc.sync.dma_start(bqk[:], bqk_d[:])
            nc.sync.dma_start(xT[:, :, ts(0, QB)], xT_d[:, :, ts(0, QB)])
            nc.sync.dma_start(wqk[:, :, 0, :], wqk_d[:, :, 0, :])
            nc.sync.dma_start(wv[:], wv_d[:])
            nc.sync.dma_start(bv_bc[:], bv_d[:].to_broadcast((P, HLOC * HD)))
            for tb in range(1, NQB):
                nc.sync.dma_start(xT[:, :, ts(tb, QB)], xT_d[:, :, ts(tb, QB)])
            for ch in (4, 1, 5, 2):
                nc.sync.dma_start(wqk[:, :, ch, :], wqk_d[:, :, ch, :])
            nc.sync.dma_start(wp[:], wp_d[:])
            nc.sync.dma_start(bp2_bc[:], bp2_d[:].to_broadcast((P, DIM)))
            nc.vector.memset(VA[:], 1.0)  # ones column; V overwritten below

            with tc.tile_pool(name="ps", bufs=1, space="PSUM") as ps:

                def qkv_tile(ch, t, tag="pq"):
                    # one [128 feat, 512 tok] output tile of Q or K
                    pq = ps.tile([P, QB], f32, tag=tag,
                                 bufs=(1 if tag == "pq" else 3),
                                 name=f"pq_{ch}_{t}")
                    for i in range(INF):
                        nc.tensor.matmul(
                            pq[:], wqk[:, i, ch, :], xT[:, i, ts(t, QB)],
                            start=(i == 0), stop=(i == INF - 1))
                    nc.vector.tensor_tensor(
                        QK[:, ch, ts(t, QB)], pq[:],
                        bqk[:, ch : ch + 1].to_broadcast((P, QB)), OP.add)

                def v_tile(tt, tag="po"):
                    pv = ps.tile([P, HLOC * HD], f32, tag=tag,
                                 bufs=(1 if tag == "pq" else 3),
                                 name=f"pv_{tt}")
                    for i in range(INF):
                        nc.tensor.matmul(
                            pv[:], xT[:, i, ts(tt, P)], wv[:, i, :],
                            start=(i == 0), stop=(i == INF - 1))
                    nc.vector.tensor_tensor(
                        VA[:, tt, :, 0:HD],
                        pv[:].rearrange("p (h d) -> p h d", d=HD),
                        bv_bc[:].rearrange("p (h d) -> p h d", d=HD),
                        OP.add)

                def normalize(j, qqb, s, po):
                    # rsum: [65, 512] PSUM -> SBUF on ACT (frees po slot);
                    # sums row redistributed SBUF->SBUF into [64, 8] for a
                    # wide reciprocal, broadcast back, multiply on GPSIMD.
                    # All DMAs ride the gpsimd queue so their waits never
                    # block the ea prefetch stream (which owns sync).
                    sl = slice(s * HD, (s + 1) * HD)
                    rsum = norm.tile([HD + 1, QB], f32, tag="rsum")
                    nc.scalar.activation(rsum[:], po[:], AF.Copy)
                    rdram = dramp.tile([1, QB], f32, tag="rdram")
                    nc.sync.dma_start(rdram[:], rsum[HD : HD + 1, :])
                    rsq = norm.tile([HD, QB // HD], f32, tag="rsq")
                    nc.sync.dma_start(
                        rsq[:], rdram[:].rearrange("o (a b) -> (o a) b", a=HD))
                    rrec = norm.tile([HD, QB // HD], bf16, tag="rrec")
                    with nc.allow_low_precision(
                            reason="bf16 softmax denominators are within "
                                   "the output tolerance"):
                        nc.vector.reciprocal(rrec[:], rsq[:])
                    rdram2 = dramp.tile([HD, QB // HD], bf16, tag="rdram2")
                    nc.sync.dma_start(rdram2[:], rrec[:])
                    rcb = norm.tile([HD, QB], bf16, tag="rcb")
                    nc.sync.dma_start(
                        rcb[:], rdram2[:].rearrange(
                            "a b -> (a b)")[None, :].to_broadcast((HD, QB)))
                    xtmp = norm.tile([HD, QB], bf16, tag="xtmp")
                    nc.gpsimd.tensor_tensor(
                        xtmp[:], rsum[0:HD, :], rcb[:], OP.mult)
                    nc.sync.dma_start(xA[sl, j, ts(qqb, QB)], xtmp[:])

                def attention_block(j, qqb, fillers, every):
                    po = [ps.tile([HD + 1, QB], f32, tag="po", bufs=3,
                                  name=f"po_{2*j+s}_{qqb}") for s in range(2)]
                    for kt in range(NKT):
                        sc = ps.tile([P, 2 * QB], f32, tag="sc", bufs=2,
                                     name=f"sc_{j}_{qqb}_{kt}")
                        for s in range(2):
                            sl = slice(s * HD, (s + 1) * HD)
                            nc.tensor.matmul(
                                sc[:, ts(s, QB)],
                                QK[sl, 3 + j, ts(kt, P)],
                                QK[sl, j, ts(qqb, QB)],
                                start=True, stop=True)
                        ea_t = stream.tile([P, 2 * QB], bf16, tag="ea",
                                           bufs=8)
                        if (j, qqb) == (0, 0) or kt % 2 == 1:
                            nc.gpsimd.dma_start(ea_t[:], ea_d[j, qqb, kt])
                        else:
                            nc.sync.dma_start(ea_t[:], ea_d[j, qqb, kt])
                        e = stream.tile([P, 2 * QB], bf16, tag="e", bufs=6)
                        nc.scalar.activation(e[:], sc[:], AF.Exp)
                        pt = stream.tile([P, 2 * QB], bf16, tag="pt", bufs=6)
                        nc.vector.tensor_tensor(pt[:], e[:], ea_t[:], OP.mult)
                        for s in range(2):
                            nc.tensor.matmul(
                                po[s][:], VA[:, kt, 2 * j + s, :],
                                pt[:, ts(s, QB)],
                                start=(kt == 0), stop=(kt == NKT - 1))
                        if kt % every == every - 1 and fillers:
                            fillers.pop(0)()
                    for s in range(2):
                        normalize(j, qqb, s, po[s])

                def proj_tile(tt):
                    # split across pq/po tags so proj never blocks the
                    # attention stream's sc slots
                    pa = ps.tile([P, QB], f32, tag="pq", bufs=1,
                                 name=f"pa_{tt}")
                    pb = ps.tile([P, DIM - QB], f32, tag="po", bufs=3,
                                 name=f"pb_{tt}")
                    for cc in range(3):
                        nc.tensor.matmul(
                            pa[:], xA[:, cc, ts(tt, P)], wp[:, cc, 0:QB],
                            start=(cc == 0), stop=(cc == 2))
                        nc.tensor.matmul(
                            pb[:], xA[:, cc, ts(tt, P)], wp[:, cc, QB:DIM],
                            start=(cc == 0), stop=(cc == 2))
                    ot = stream.tile([P, DIM], bf16, tag="ot")
                    nc.vector.tensor_tensor(
                        ot[:, 0:QB], pa[:], bp2_bc[:, 0:QB], OP.add)
                    nc.vector.tensor_tensor(
                        ot[:, QB:DIM], pb[:], bp2_bc[:, QB:DIM], OP.add)
                    nc.sync.dma_start(out_d[ts(tt, P), :], ot[:])

                # ---- program order ----
                # startup: K0 tile0 + Q0 tile0 unblock the first scores;
                # remaining K0 tiles + first v tiles follow, alternating
                # psum tags so they pipeline 2-deep
                qkv_tile(3, 0, tag="pq")
                qkv_tile(0, 0, tag="po")
                for i, t in enumerate(range(1, NQB)):
                    qkv_tile(3, t, tag=("pq" if i % 2 == 0 else "po"))
                for tt in range(4):
                    v_tile(tt, tag=("pq" if tt % 2 == 0 else "po"))

                # filler schedule per (j, qqb): Q chunk tile t must land
                # before block qqb=t of its pair; K chunk tile t before kt=4t
                # of its pair's j-phase; proj tile tt after j2-block tt//4.
                def QT(ch, t):
                    return lambda: qkv_tile(ch, t)

                def VT(tt):
                    return lambda: v_tile(tt)

                def PT(tt):
                    return lambda: proj_tile(tt)

                sched = {
                    (0, 0): ([VT(tt) for tt in range(4, NTT)] + [QT(0, 1)], 1),
                    (0, 1): ([QT(0, 2), QT(4, 0), QT(4, 1), QT(4, 2)], 4),
                    (0, 2): ([QT(0, 3), QT(4, 3), QT(1, 0), QT(1, 1)], 4),
                    (0, 3): ([QT(1, 2), QT(1, 3)], 8),
                    (1, 0): ([QT(5, 0), QT(5, 1), QT(5, 2), QT(5, 3)], 4),
                    (1, 1): ([QT(2, 0), QT(2, 1), QT(2, 2), QT(2, 3)], 4),
                    (1, 2): ([], 16),
                    (1, 3): ([], 16),
                    (2, 0): ([], 16),
                    (2, 1): ([PT(0), PT(1), PT(2), PT(3)], 4),
                    (2, 2): ([PT(4), PT(5), PT(6), PT(7)], 4),
                    (2, 3): ([PT(8), PT(9), PT(10), PT(11)], 4),
                }
                for j in range(3):
                    for qqb in range(NQB):
                        fillers, every = sched[(j, qqb)]
                        attention_block(j, qqb, fillers, every)
                for tt in range(12, 16):
                    proj_tile(tt)

    nc.finalize()
    return nc


def _get_nc():
    global _CACHED_NC
    if _CACHED_NC is None:
        _CACHED_NC = _build()
    return _CACHED_NC


def _make_sharded(nc, n_cores=8, donate=False):
    """jit-wrapped shard_map over the prebuilt Bass module."""
    import jax
    from jax.sharding import Mesh, PartitionSpec
    from jax.experimental.shard_map import shard_map
    from concourse import bass2jax

    bass2jax.install_neuronx_cc_hook()
    partition_name = (nc.partition_id_tensor.name if nc.partition_id_tensor
                      else None)
    in_names, out_names, out_avals, zero_outs = [], [], [], []
    for alloc in nc.m.functions[0].allocations:
        if not isinstance(alloc, mybir.MemoryLocationSet):
            continue
        name = alloc.memorylocations[0].name
        if alloc.kind == "ExternalInput":
            if name != partition_name:
                in_names.append(name)
        elif alloc.kind == "ExternalOutput":
            out_names.append(name)
            shape = tuple(alloc.tensor_shape)
            dtype = mybir.dt.np(alloc.dtype)
            out_avals.append(jax.core.ShapedArray(shape, dtype))
            zero_outs.append(np.zeros(shape, dtype))
    n_params = len(in_names)
    n_outs = len(out_avals)
    all_in_names = list(in_names) + list(out_names)
    if partition_name is not None:
        all_in_names.append(partition_name)

    def _body(*args):
        operands = list(args)
        if partition_name is not None:
            operands.append(bass2jax.partition_id_tensor())
        outs = bass2jax._bass_exec_p.bind(
            *operands,
            out_avals=tuple(out_avals),
            in_names=tuple(all_in_names),
            out_names=tuple(out_names),
            lowering_input_output_aliases=(),
            sim_require_finite=True,
            sim_require_nnan=True,
            nc=nc,
        )
        return tuple(outs)

    devices = jax.devices()[:n_cores]
    mesh = Mesh(np.asarray(devices), ("core",))
    in_specs = (PartitionSpec("core"),) * (n_params + n_outs)
    out_specs = (PartitionSpec("core"),) * len(out_names)
    kw = dict(keep_unused=True)
    if donate:
        kw["donate_argnums"] = tuple(range(n_params, n_params + n_outs))
    sharded = jax.jit(
        shard_map(_body, mesh=mesh, in_specs=in_specs, out_specs=out_specs,
                  check_rep=False), **kw)
    return sharded, mesh, in_names, out_names, zero_outs, n_params


def _prep_inputs(inputs, mask, alibi_bias, Wqkv, bqkv, Wproj, bproj):
    """Build the 8 per-core input maps (host-side sharding / layout prep)."""
    inputs = np.asarray(inputs, dtype=np.float32)
    mask = np.asarray(mask)
    alibi_bias = np.asarray(alibi_bias, dtype=np.float32)
    Wqkv = np.asarray(Wqkv, dtype=np.float32)
    bqkv = np.asarray(bqkv, dtype=np.float32)
    Wproj = np.asarray(Wproj, dtype=np.float32)
    bproj = np.asarray(bproj, dtype=np.float32)
    bf = ml_dtypes.bfloat16

    mask_uniform = bool(mask.all())

    def ea_group(hs, b):
        # exp(alibi^T) packed as [j, qqb, kt, 128 k, h0 512 q | h1 512 q]
        a = alibi_bias[0, hs : hs + HLOC].transpose(0, 2, 1)  # [6, k, q]
        if not mask_uniform:
            mb = np.where(mask[b], 0.0, -1e30).astype(np.float32)  # [S] on k
            a = a + mb[None, :, None]
        ea = np.exp(a, dtype=np.float32).astype(bf)  # [6, k, q]
        out = np.empty((3, NQB, NKT, P, 2 * QB), dtype=bf)
        for j in range(3):
            for s in range(2):
                h = 2 * j + s
                for qqb in range(NQB):
                    for kt in range(NKT):
                        out[j, qqb, kt, :, s * QB : (s + 1) * QB] = (
                            ea[h, kt * P : (kt + 1) * P,
                               qqb * QB : (qqb + 1) * QB])
        return out

    if mask_uniform:
        ea_cache = {0: ea_group(0, 0), HLOC: ea_group(HLOC, 0)}
    else:
        ea_cache = {}

    def core_weights(hs):
        wqk = np.empty((P, INF, 6, P), dtype=np.float32)
        bqk = np.empty((P, 6), dtype=np.float32)
        W3 = Wqkv.reshape(INF, P, H, 3, HD)
        b3 = bqkv.reshape(H, 3, HD)
        Wq, Wk = W3[:, :, :, 0, :], W3[:, :, :, 1, :]
        bq, bk = b3[:, 0, :], b3[:, 1, :]
        for j in range(3):
            for s in range(2):
                h = hs + 2 * j + s
                wqk[:, :, j, s * HD : (s + 1) * HD] = (
                    Wq[:, :, h] * SCALE).transpose(1, 0, 2)
                wqk[:, :, 3 + j, s * HD : (s + 1) * HD] = (
                    Wk[:, :, h]).transpose(1, 0, 2)
                bqk[s * HD : (s + 1) * HD, j] = bq[h] * SCALE
                bqk[s * HD : (s + 1) * HD, 3 + j] = bk[h]
        Wv = W3[:, :, :, 2, :]
        wv = np.ascontiguousarray(
            Wv[:, :, hs : hs + HLOC].transpose(1, 0, 2, 3).reshape(
                P, INF, HLOC * HD), dtype=bf)
        bv = np.ascontiguousarray(
            b3[hs : hs + HLOC, 2, :].reshape(1, HLOC * HD), dtype=np.float32)
        wp = np.ascontiguousarray(
            Wproj[hs * HD : (hs + HLOC) * HD].reshape(3, P, DIM).transpose(
                1, 0, 2), dtype=bf)
        return (np.ascontiguousarray(wqk, dtype=bf), bqk, wv, bv, wp)

    bp2 = (bproj[None, :] * 0.5).astype(np.float32)
    wcache = {0: core_weights(0), HLOC: core_weights(HLOC)}

    in_maps = []
    for c in range(8):
        b = c // 2
        hs = HLOC * (c % 2)
        xT = np.ascontiguousarray(
            inputs[b].T.reshape(INF, P, S).transpose(1, 0, 2), dtype=bf)
        wqk, bqk, wv, bv, wp = wcache[hs]
        ea = ea_cache[hs] if mask_uniform else ea_group(hs, b)
        in_maps.append({
            "xT": xT, "wqk": wqk, "bqk": bqk, "wv": wv, "bv": bv,
            "wp": wp, "bp2": bp2, "ea": ea,
        })
    return in_maps


def _run(in_maps, trace=False):
    nc = _get_nc()
    return run_bass_kernel_spmd(nc, in_maps, core_ids=list(range(8)),
                                trace=trace)


def _assemble(results):
    out = np.empty((B, S, DIM), dtype=np.float32)
    for b in range(B):
        out[b] = (results[2 * b]["out"].astype(np.float32)
                  + results[2 * b + 1]["out"].astype(np.float32))
    return out


def kernel(inputs, mask, alibi_bias, Wqkv, bqkv, Wproj, bproj):
    in_maps = _prep_inputs(inputs, mask, alibi_bias, Wqkv, bqkv, Wproj, bproj)
    res = _run(in_maps, trace=False)
    return _assemble(res.results)


def kernel_traced(inputs, mask, alibi_bias, Wqkv, bqkv, Wproj, bproj,
                  samples=5):
    """Like kernel() but also returns neuron-profile exec time in ns."""
    import ctypes
    import tempfile

    import jax
    from jax.sharding import NamedSharding, PartitionSpec

    from concourse._compat import FishPath
    from gauge.profiler import Profile

    lib = ctypes.CDLL("/opt/axon/libaxon_pjrt.so")
    lib.axon_start_nrt_profile.argtypes = [ctypes.c_char_p, ctypes.c_size_t]
    lib.axon_start_nrt_profile.restype = ctypes.c_int64
    lib.axon_stop_nrt_profile.argtypes = [ctypes.c_char_p, ctypes.c_size_t]
    lib.axon_stop_nrt_profile.restype = ctypes.c_int64

    in_maps = _prep_inputs(inputs, mask, alibi_bias, Wqkv, bqkv, Wproj, bproj)
    nc = _get_nc()
    sharded, mesh, in_names, out_names, zero_outs, n_params = _make_sharded(
        nc, 8)
    sh = NamedSharding(mesh, PartitionSpec("core"))
    dev_in = [jax.device_put(
        np.concatenate([np.asarray(in_maps[c][k]) for c in range(8)], axis=0),
        sh) for k in in_names]
    dev_zero = [jax.device_put(
        np.zeros((8 * z.shape[0], *z.shape[1:]), z.dtype), sh)
        for z in zero_outs]
    out = sharded(*dev_in, *dev_zero)
    jax.block_until_ready(out)

    times = []
    last_dir = None
    for _ in range(samples):
        outdir = tempfile.mkdtemp(prefix="ntff_")
        d = outdir.encode()
        if lib.axon_start_nrt_profile(d, len(d)) != 0:
            break
        out = sharded(*dev_in, *dev_zero)
        jax.block_until_ready(out)
        lib.axon_stop_nrt_profile(d, len(d))
        try:
            prof = Profile(profile_path=FishPath(outdir),
                           kernel_dev_mode=True, profile_on_exit=False,
                           bass_kernel=nc.m, offline_processing=True,
                           fname="*_body*")
            res = prof.to_perfetto(model_index=(0,))
            if res and res[0].exec_time_ns:
                times.append(res[0].exec_time_ns)
                last_dir = outdir
        except Exception:
            pass
    if last_dir:
        print(f"profile dir: {last_dir}")

    i = out_names.index("out")
    arr = np.asarray(out[i]).reshape(8, S, DIM).astype(np.float32)
    full = np.empty((B, S, DIM), np.float32)
    for b in range(B):
        full[b] = arr[2 * b] + arr[2 * b + 1]
    return full, (min(times) if times else None)
